# revision 2
# baseline (speedup 1.0000x reference)
"""Multi-head attention (B=2, S=2048, D=1024, H=16, HD=64) on 8 TRN2 NeuronCores.

v2: all-bf16 operands, single fine-grained software-pipelined schedule.
Sharding: core c -> (batch b = c//4, head-group g = c%4 of 4 heads).

Per-core dataflow (identical math to v1):
  X^T [d, s] bf16; Q^T,K^T in [hd, s] (2 heads per 128-partition tile);
  V in [s, hd] with a mask-scaled ones column per (head, key-chunk) so the
  PV matmul yields softmax denominators for free.  scores^T = K^T-stationary
  matmul in [k, q] layout; exp on ACT straight out of PSUM (no max-subtract:
  scores ~N(0, 0.4^2)); ctx^T normalized by 1/den after PV; Y = ctx^T.T @ Wo.

  Schedule: one global stream.  Steady state per unit (h, jb) and per
  chunk-group cg: [sc(u+1, cg) x2 mm, exp(u+1, cg), pv(u, cg) x2 mm,
  filler mm...] where filler = Q/K/V projection groups and Y output tiles,
  demand-pulled ahead of their first consumer and otherwise paced evenly to
  keep the PE stream dense (the PE p-state drops on idle gaps, halving
  matmul throughput, so PE density is the whole game).
"""

import numpy as np

B, S, D = 2, 2048, 1024
H, HD = 16, 64
NCORES = 8
HG = 4             # heads per core
HDG = HG * HD      # 256 head-dims per core
P = 128
KC = S // P        # 16 key chunks
DC = D // P        # 8 contraction chunks for projections
NQ = 512           # q-block size
NJ = S // NQ       # 4 q-blocks
SCK = 2            # score psum tile spans 2 key-chunks -> [128, 1024]
NCG = KC // SCK    # 8 chunk-groups per unit
VW = HD + 1        # 65: V columns + ones column


def _build_program(reps=1):
    import concourse.mybir as mybir
    import concourse.tile as tile
    from concourse import bacc

    fp32 = mybir.dt.float32
    bf16 = mybir.dt.bfloat16
    Act = mybir.ActivationFunctionType

    nc = bacc.Bacc("TRN2", target_bir_lowering=False, debug=False,
                   num_devices=NCORES)

    XT = nc.dram_tensor("XT", [D, S], bf16, kind="ExternalInput").ap()
    WQ = nc.dram_tensor("WQ", [D, HDG], bf16, kind="ExternalInput").ap()
    WK = nc.dram_tensor("WK", [D, HDG], bf16, kind="ExternalInput").ap()
    WV = nc.dram_tensor("WV", [D, HDG], bf16, kind="ExternalInput").ap()
    WO = nc.dram_tensor("WO", [HDG, D], bf16, kind="ExternalInput").ap()
    BQ = nc.dram_tensor("BQ", [P, 2], fp32, kind="ExternalInput").ap()
    BK = nc.dram_tensor("BK", [P, 2], fp32, kind="ExternalInput").ap()
    MASKT = nc.dram_tensor("MASKT", [P, KC], fp32, kind="ExternalInput").ap()
    Y = nc.dram_tensor("Y", [S, D], bf16, kind="ExternalOutput").ap()

    from contextlib import ExitStack
    with tile.TileContext(nc) as tc, ExitStack() as _loop_stk, \
         tc.tile_pool(name="persist", bufs=1) as persist, \
         tc.tile_pool(name="expt_pool", bufs=3) as exptp, \
         tc.tile_pool(name="small", bufs=2) as small, \
         tc.tile_pool(name="yout", bufs=3) as yout, \
         tc.tile_pool(name="ctxp", bufs=4) as ctxp, \
         tc.tile_pool(name="scps", bufs=2, space="PSUM") as scps, \
         tc.tile_pool(name="pvps", bufs=2, space="PSUM") as pvps, \
         tc.tile_pool(name="miscps", bufs=2, space="PSUM") as miscps:

        if reps > 1:
            import concourse.mybir as _mb
            _loop_stk.enter_context(tc.For_i(
                0, reps, 1,
                hint_engines=(_mb.EngineType.PE, _mb.EngineType.Activation,
                              _mb.EngineType.DVE, _mb.EngineType.SP,
                              _mb.EngineType.Pool)))

        bq_sb = persist.tile([P, 2], fp32)
        bk_sb = persist.tile([P, 2], fp32)
        mask_sb = persist.tile([P, KC], fp32)

        qt_sb = [persist.tile([P, S], bf16, name=f"qt{i}") for i in range(2)]
        kt_sb = [persist.tile([P, S], bf16, name=f"kt{i}") for i in range(2)]
        vaug = persist.tile([P, HG * KC * VW], bf16)
        wo_sb = persist.tile([P, 2 * D], bf16)
        wq_sb = persist.tile([P, DC * HDG], bf16)
        wk_sb = persist.tile([P, DC * HDG], bf16)
        wv_sb = persist.tile([P, DC * HDG], bf16)
        xt = [persist.tile([P, S], bf16, name=f"xt{c}") for c in range(DC)]

        # ---------- DMA emission (arrival order = first-use order) ----------
        # Inputs split across BOTH hwdge queues: weights/biases on the ACT
        # queue, X^T on the SP queue, so the two streams transfer in
        # parallel and the pre-phase isn't gated on one serial DMA chain.
        nc.sync.dma_start(bq_sb[:], BQ[:])
        nc.sync.dma_start(bk_sb[:], BK[:])
        nc.sync.dma_start(mask_sb[:], MASKT[:])

        def load_w(w_sb, W, c):
            nc.scalar.dma_start(w_sb[:, c * HDG:(c + 1) * HDG],
                                W[c * P:(c + 1) * P, :])

        def load_xt(c, jb):
            nc.sync.dma_start(
                xt[c][:, jb * NQ:(jb + 1) * NQ],
                XT[c * P:(c + 1) * P, jb * NQ:(jb + 1) * NQ])

        for c in range(DC):
            load_w(wq_sb, WQ, c)
            load_xt(c, 0)
        for c in range(DC):
            load_w(wk_sb, WK, c)
        for c in range(DC):
            load_w(wv_sb, WV, c)
        for c in range(2):
            nc.scalar.dma_start(wo_sb[:, c * D:(c + 1) * D],
                                WO[c * P:(c + 1) * P, :])
        for jb in range(1, NJ):
            for c in range(DC):
                load_xt(c, jb)
        # warm the exp table after the ACT-queue DMA issues (engine work
        # would otherwise delay those issues on ACT's in-order sequencer)
        warm = persist.tile([1, 1], fp32)
        nc.scalar.activation(warm[:], bq_sb[0:1, 0:1], Act.Exp)

        # ---------- emission helpers ----------
        expts = {}
        pvs = {}
        ctxts = {}

        def emit_scores_cg(h, jb, cg):
            hc, hp = h // 2, (h % 2) * 64
            if (h, jb) not in expts:
                expts[(h, jb)] = exptp.tile([P, KC * NQ], bf16, tag="expt",
                                            name=f"expt_{h}_{jb}")
            expt = expts[(h, jb)]
            sc = scps.tile([P, SCK * NQ], fp32, tag="sc",
                           name=f"sc_{h}_{jb}_{cg}")
            for u in range(SCK):
                c = cg * SCK + u
                nc.tensor.matmul(
                    sc[:, u * NQ:(u + 1) * NQ],
                    kt_sb[hc][hp:hp + 64, c * P:(c + 1) * P],
                    qt_sb[hc][hp:hp + 64, jb * NQ:(jb + 1) * NQ],
                    start=True, stop=True)
            nc.scalar.activation(
                expt[:, cg * SCK * NQ:(cg + 1) * SCK * NQ], sc[:], Act.Exp)

        def pv_begin(h, jb):
            pvs[(h, jb)] = pvps.tile([VW, NQ], fp32, tag="pv",
                                     name=f"pv_{h}_{jb}")

        def pv_cg(h, jb, cg):
            pv = pvs[(h, jb)]
            expt = expts[(h, jb)]
            for u in range(SCK):
                c = cg * SCK + u
                nc.tensor.matmul(
                    pv[:],
                    vaug[:, (h * KC + c) * VW:(h * KC + c + 1) * VW],
                    expt[:, c * NQ:(c + 1) * NQ],
                    start=(c == 0), stop=(c == KC - 1))

        def pv_end(h, jb, pe_bcast=False):
            hc, hp = h // 2, (h % 2) * 64
            expts.pop((h, jb))
            pv = pvs.pop((h, jb))
            if (hc, jb) not in ctxts:
                ctxts[(hc, jb)] = ctxp.tile([P, NQ], bf16, tag=f"ct{hc}",
                                            name=f"ctxt_{hc}_{jb}")
            ct = ctxts[(hc, jb)]
            r64 = small.tile([VW, NQ], fp32, tag="r64", bufs=1)
            nc.vector.reciprocal(r64[64:65, :], pv[64:65, :])
            # row-shift the recip into this tile's unused row 0
            nc.sync.dma_start(r64[0:1, :], r64[64:65, :])
            rb = small.tile([64, NQ], fp32, tag="rb", bufs=2)
            nc.gpsimd.partition_broadcast(rb[:], r64[0:1, :])
            if hp == 0:
                nc.vector.tensor_mul(ct[0:64, :], pv[0:64, :], rb[:])
            else:
                stg = small.tile([64, NQ], bf16, tag="stg", bufs=2)
                nc.vector.tensor_mul(stg[:], pv[0:64, :], rb[:])
                nc.sync.dma_start(ct[64:128, :], stg[:])

        # ---------- filler steps (single-matmul granularity) ----------
        from collections import deque
        filler = deque()
        qk_done = {}     # ("q"|"k", hc, jb) -> bool
        v_done = [False] * KC

        def proj_steps(kind, w_sb, t_sb, b_sb, hc, jb):
            tag = f"{kind}{hc}{jb}"
            holder = {}

            def step(c, tag=tag, holder=holder, w_sb=w_sb, t_sb=t_sb,
                     b_sb=b_sb, hc=hc, jb=jb, kind=kind):
                if c == 0:
                    holder["pp"] = miscps.tile([P, NQ], fp32, tag="mp",
                                               name=f"pp_{tag}")
                pp = holder["pp"]
                nc.tensor.matmul(
                    pp[:],
                    w_sb[:, c * HDG + hc * P:c * HDG + (hc + 1) * P],
                    xt[c][:, jb * NQ:(jb + 1) * NQ],
                    start=(c == 0), stop=(c == DC - 1))
                if c == DC - 1:
                    nc.vector.tensor_scalar(
                        t_sb[hc][:, jb * NQ:(jb + 1) * NQ], pp[:],
                        b_sb[:, hc:hc + 1], None, mybir.AluOpType.add)
                    qk_done[(kind, hc, jb)] = True
            return [lambda c=c: step(c) for c in range(DC)]

        def v_steps(i):
            holder = {}

            def step(c, i=i, holder=holder):
                if c == 0:
                    holder["vp"] = miscps.tile([P, NQ], fp32, tag="mp",
                                               name=f"vp_{i}")
                vp = holder["vp"]
                nc.tensor.matmul(
                    vp[:, 0:HDG], xt[c][:, i * P:(i + 1) * P],
                    wv_sb[:, c * HDG:(c + 1) * HDG],
                    start=(c == 0), stop=(c == DC - 1))
                if c == DC - 1:
                    for h in range(HG):
                        oc = (h * KC + i) * VW + HD
                        nc.vector.tensor_copy(vaug[:, oc:oc + 1],
                                              mask_sb[:, i:i + 1])
                    for h in range(HG):
                        vc = (h * KC + i) * VW
                        nc.vector.tensor_scalar(
                            vaug[:, vc:vc + HD], vp[:, h * HD:(h + 1) * HD],
                            mask_sb[:, i:i + 1], None, mybir.AluOpType.mult)
                    v_done[i] = True
            return [lambda c=c: step(c) for c in range(DC)]

        def y_steps(jb):
            c0, c1 = ctxts.pop((0, jb)), ctxts.pop((1, jb))
            steps = []
            for m in range(jb * NQ // P, (jb + 1) * NQ // P):
                mo = (m - jb * NQ // P) * P
                for dh in range(2):
                    holder = {}

                    def s0(m=m, mo=mo, dh=dh, holder=holder):
                        holder["yp"] = miscps.tile([P, 512], fp32, tag="mp",
                                                   name=f"yp_{m}_{dh}")
                        nc.tensor.matmul(
                            holder["yp"][:], c0[:, mo:mo + P],
                            wo_sb[:, dh * 512:(dh + 1) * 512],
                            start=True, stop=False)

                    def s1(m=m, mo=mo, dh=dh, holder=holder):
                        yp = holder["yp"]
                        nc.tensor.matmul(
                            yp[:], c1[:, mo:mo + P],
                            wo_sb[:, D + dh * 512:D + (dh + 1) * 512],
                            start=False, stop=True)
                        ysb = yout.tile([P, 512], bf16, tag="ysb",
                                        name=f"ysb_{m}_{dh}")
                        nc.vector.tensor_copy(ysb[:], yp[:])
                        nc.sync.dma_start(
                            Y[m * P:(m + 1) * P, dh * 512:(dh + 1) * 512],
                            ysb[:])
                    steps.append(s0)
                    steps.append(s1)
            return steps

        def pop_filler(n):
            for _ in range(n):
                if filler:
                    filler.popleft()()

        def ensure(kind, hc, jb):
            while not qk_done.get((kind, hc, jb)):
                if not filler:
                    raise RuntimeError(f"filler underflow: {kind}{hc}{jb}")
                filler.popleft()()

        def ensure_v(i):
            while not v_done[min(i, KC - 1)]:
                if not filler:
                    raise RuntimeError("filler underflow in ensure_v")
                filler.popleft()()

        # ---------- fill the filler queue ----------
        # Scores for unit (h, jb) read qt[hc] slice jb but kt[hc] over ALL
        # key chunks (keys span the full sequence): kt[hc] must be complete
        # before that unit's score stream.  Queue order reflects first use;
        # ensure() demand-pulls ahead of each consumer, pop_filler paces the
        # rest to keep PE dense.
        # jb3 runs heads (3,2,1,0) so the final unit's pv_end is an even
        # head (direct DVE write, no staging DMA in the tail chain)
        units = [(h, jb) for jb in range(NJ)
                 for h in ((3, 2, 1, 0) if jb == NJ - 1 else (0, 1, 2, 3))]

        filler.extend(proj_steps("q", wq_sb, qt_sb, bq_sb, 0, 0))
        for jb in range(NJ):
            filler.extend(proj_steps("k", wk_sb, kt_sb, bk_sb, 0, jb))
        for i in range(KC):
            filler.extend(v_steps(i))
        filler.extend(proj_steps("q", wq_sb, qt_sb, bq_sb, 1, 0))
        for jb in range(NJ):
            filler.extend(proj_steps("k", wk_sb, kt_sb, bk_sb, 1, jb))
        for jb in range(1, NJ):
            filler.extend(proj_steps("q", wq_sb, qt_sb, bq_sb, 0, jb))
        for jb in range(1, NJ):
            filler.extend(proj_steps("q", wq_sb, qt_sb, bq_sb, 1, jb))

        KJB = NQ // P  # key chunks per kt jb-slice (4)

        def emit_unit_scores(h, jb, cg):
            hc = h // 2
            ensure("q", hc, jb)
            # prefetch kt two chunk-groups ahead (hide proj->bias latency)
            kjb = min((SCK * (cg + 2) + SCK - 1) // KJB, NJ - 1)
            ensure("k", hc, kjb)
            emit_scores_cg(h, jb, cg)

        # ---------- pre-phase: unit 0 projections + scores ----------
        ensure_v(3)
        for cg in range(NCG):
            emit_unit_scores(0, 0, cg)
            ensure_v(2 * cg + 3)
            pop_filler(1)

        # ---------- main unit loop ----------
        total_slots = len(units) * NCG
        slot = 0
        pending_y = None
        for ui, (h, jb) in enumerate(units):
            nxt = units[ui + 1] if ui + 1 < len(units) else None
            if pending_y is not None:
                filler.extend(y_steps(pending_y))
                pending_y = None
            pv_begin(h, jb)
            if nxt is None:
                # last unit: dense pv stream so its stop (and the normalize
                # chain) lands as early as possible; leftover filler then
                # covers the chain latency before the final Y block
                for cg in range(NCG):
                    pv_cg(h, jb, cg)
                while filler:
                    filler.popleft()()
            else:
                for cg in range(NCG):
                    ensure_v(2 * cg + 3)
                    emit_unit_scores(nxt[0], nxt[1], cg)
                    pv_cg(h, jb, cg)
                    remaining = total_slots - slot
                    n = (len(filler) + remaining - 1) // remaining
                    pop_filler(min(n, 3))
                    slot += 1
            pv_end(h, jb)
            if (h, jb) in ((HG - 1, 0), (HG - 1, 1), (HG - 1, 2), (0, NJ - 1)):
                pending_y = jb
        if pending_y is not None:
            filler.extend(y_steps(pending_y))
        while filler:
            filler.popleft()()

    nc.finalize()
    return nc


_program_cache = {}


def _get_program():
    if "nc" not in _program_cache:
        _program_cache["nc"] = _build_program()
    return _program_cache["nc"]


def _to_bf16(a):
    import ml_dtypes
    return np.ascontiguousarray(np.asarray(a, np.float32)).astype(
        ml_dtypes.bfloat16)


def _make_in_maps(inputs):
    X = np.asarray(inputs["X"], np.float32)
    mask = np.asarray(inputs["mask"], np.float32)
    Wq = np.asarray(inputs["Wq"], np.float32)
    Wk = np.asarray(inputs["Wk"], np.float32)
    Wv = np.asarray(inputs["Wv"], np.float32)
    Wo = np.asarray(inputs["Wo"], np.float32)
    bq = np.asarray(inputs["bq"], np.float32)
    bk = np.asarray(inputs["bk"], np.float32)

    scale = np.float32(1.0 / np.sqrt(HD))
    in_maps = []
    for c in range(NCORES):
        b, g = c // HG, c % HG
        sl = slice(g * HDG, (g + 1) * HDG)
        in_maps.append({
            "XT": _to_bf16(X[b].T),
            "WQ": _to_bf16(Wq[:, sl] * scale),
            "WK": _to_bf16(Wk[:, sl]),
            "WV": _to_bf16(Wv[:, sl]),
            "WO": _to_bf16(Wo[sl, :]),
            "BQ": np.ascontiguousarray((bq[sl] * scale).reshape(2, P).T),
            "BK": np.ascontiguousarray(bk[sl].reshape(2, P).T),
            "MASKT": np.ascontiguousarray(mask[b].reshape(KC, P).T),
        })
    return in_maps


def _run(inputs, trace=False, tmpdir=None):
    from concourse import bass_utils

    nc = _get_program()
    in_maps = _make_in_maps(inputs)
    res = bass_utils.run_bass_kernel_spmd(
        nc, in_maps, core_ids=list(range(NCORES)), trace=trace, tmpdir=tmpdir)

    bv = np.asarray(inputs["bv"], np.float32)
    bo = np.asarray(inputs["bo"], np.float32)
    Wo = np.asarray(inputs["Wo"], np.float32)
    row = bv @ Wo + bo  # exact bv/bo contribution (attn rows sum to 1)

    out = np.zeros((B, S, D), np.float32)
    for c in range(NCORES):
        out[c // HG] += np.asarray(res.results[c]["Y"], np.float32)
    out += row[None, None, :]
    return out, res


def kernel(**inputs):
    out, _ = _run(inputs, trace=False)
    return out
